# revision 39
# baseline (speedup 1.0000x reference)
"""AttributeMemoryFusion kernel for 8x TRN2 NeuronCores (Bass/Tile), v7.

Per-sample attention over ragged memory + gated fusion:
    scores = mem @ h ; attn = softmax(mask(scores)) ; r = attn @ mem
    g = sigmoid(h @ Wg.T + r @ Ug.T + b) ; out = where(len>0, g*r+(1-g)*h, h)

v7 = v6 (length-sorted ragged packing) + int8 row-quantized mem transport.
  The wall-clock of a kernel() call here is dominated by the ~80 MB/s axon
  host->device tunnel, so mem rows are shipped as int8 `q` with a per-row
  f16 `scale` (q = round(mem_row / scale), scale = absmax/127) instead of
  bf16 — halving the dominant payload. On device q is cast once per tile to
  bf16 (integers <= 127 are exact in bf16); `scale` is folded into the
  scores before the softmax and into the exp weights before the attn @ mem
  matmul, so no per-row dequantization pass is needed and the compute
  pipeline is unchanged from v6. The output returns as bf16 to halve the
  device->host payload too.

v7.3 consolidates the wire format to 3 input tensors (mem int8;
  scales+lengths f16; h+Wg+Ug+bias bf16) — each separate array shipped
  through the tunnel costs ~50-75ms of fixed overhead on top of bytes/BW,
  so folding 8 inputs into 3 saves ~0.3s/call.

v7.2 adds per-GROUP row caps: partitions within a tile are length-sorted,
  so each group of GP=16 partitions gets its own cap (the max length of its
  128 globally-consecutive sorted samples — identical on every core, so the
  SPMD program is still uniform). Each tile's mem/scales arrive as NG=8
  group DMAs instead of 1, cutting the shipped padding from ~12% to ~2.5%.
  SBUF rows above a group's cap are left as whatever the pool buffer held
  (int8 garbage casts to finite bf16) and are neutralized by zeroing the
  scales tile first: scale==0 rows get score 0 -> masked to -BIG, and
  exp-weight 0 in the r matmul.
"""

import os
import tempfile
from contextlib import ExitStack

import numpy as np
import ml_dtypes

import jax

import concourse.bacc as bacc
import concourse.mybir as mybir
import concourse.tile as tile
from concourse import masks
from concourse.bass_utils import run_bass_kernel_spmd

# run_bass_kernel_spmd rebuilds its jax.jit(shard_map(...)) wrapper on every
# call, so without a persistent compilation cache each kernel() call pays a
# fresh XLA compile (~0.4s here). The cache makes repeat calls hit disk.
try:
    jax.config.update(
        "jax_compilation_cache_dir",
        os.path.join(tempfile.gettempdir(), "jax_comp_cache"))
    jax.config.update("jax_persistent_cache_min_entry_size_bytes", -1)
    jax.config.update("jax_persistent_cache_min_compile_time_secs", 0.0)
except Exception:
    pass

B, M, D = 8192, 64, 256
N_CORES = 8
BC = B // N_CORES      # samples per core
P = 128                # partitions / samples per tile
N_TILES = BC // P
GP = 16                # partitions per cap group
NG = P // GP           # cap groups per tile
BIG = 1.0e9
REPS = 1  # profiling knob; >1 repeats the device pipeline to expose exec time

F32 = mybir.dt.float32
F16 = mybir.dt.float16
BF16 = mybir.dt.bfloat16
I8 = mybir.dt.int8
Alu = mybir.AluOpType
Act = mybir.ActivationFunctionType
AX = mybir.AxisListType


def _build_body(ctx, tc, io, caps):
    nc = tc.nc
    hw_ap, mem_ap, sc_ap, out_ap = io
    # hw_ap rows: [0:BC) = h, [BC:BC+D) = Wg, [BC+D:BC+2D) = Ug,
    # [BC+2D] = summed gate bias. sc_ap: [0:SROWS) = packed row scales,
    # [SROWS:SROWS+BC) = lengths as f16 (<= 64, exact).
    # flat packed-row offset of (tile, group)
    goffs = np.concatenate(
        [[0], np.cumsum([GP * c for row in caps for c in row])]
    ).reshape(-1)
    tile_caps = [max(row) for row in caps]
    SROWS = int(goffs[-1])

    # ---- one-time constants ----
    const = ctx.enter_context(tc.tile_pool(name="const", bufs=1))
    ident = const.tile([P, P], F32)
    masks.make_identity(nc, ident[:])
    iota_m = const.tile([P, M], F32)
    nc.gpsimd.iota(
        iota_m[:], pattern=[[1, M]], base=0, channel_multiplier=0,
        allow_small_or_imprecise_dtypes=True,
    )
    ident16 = const.tile([P, P], BF16)
    nc.vector.tensor_copy(ident16[:], ident[:])

    # ---- weights (shipped bf16): load natural [o,i], transpose to lhsT
    # layout [i_in, i_blk, o] ----
    wpool = ctx.enter_context(tc.tile_pool(name="weights", bufs=1))
    wg_nat = wpool.tile([P, 2, D], BF16)
    ug_nat = wpool.tile([P, 2, D], BF16)
    nc.sync.dma_start(
        wg_nat[:], hw_ap[BC:BC + D, :].rearrange("(a p) i -> p a i", p=P))
    nc.sync.dma_start(
        ug_nat[:], hw_ap[BC + D:BC + 2 * D, :].rearrange("(a p) i -> p a i", p=P))
    wgT = wpool.tile([P, 2, D], BF16)
    ugT = wpool.tile([P, 2, D], BF16)
    with tc.tile_pool(name="psw", bufs=2, space="PSUM") as psw:
        for nat, T in ((wg_nat, wgT), (ug_nat, ugT)):
            for ob in range(2):
                for ib in range(2):
                    pt = psw.tile([P, P], BF16, tag="wtr")
                    nc.tensor.transpose(pt[:], nat[:, ob, ib * P:(ib + 1) * P], ident16[:])
                    nc.scalar.copy(T[:, ib, ob * P:(ob + 1) * P], pt[:])

    # pre-summed (host) gate bias as a [1, D] bf16 row; added to the
    # [b, o]-layout gate preactivation via a rank-1 matmul (ones x bias_row)
    bias_row = wpool.tile([1, D], BF16)
    nc.sync.dma_start(bias_row[:], hw_ap[BC + 2 * D:BC + 2 * D + 1, :])
    ones_col = wpool.tile([1, P], BF16)
    nc.vector.memset(ones_col[:], 1.0)
    ones_D = wpool.tile([1, D], BF16)
    nc.vector.memset(ones_D[:], 1.0)

    # ---- pools ----
    memq_pool = ctx.enter_context(tc.tile_pool(name="memq", bufs=3))
    mem_pool = ctx.enter_context(tc.tile_pool(name="mem", bufs=3))
    small = ctx.enter_context(tc.tile_pool(name="small", bufs=3))
    xstage = ctx.enter_context(tc.tile_pool(name="xstage", bufs=3))
    diag_pool = ctx.enter_context(tc.tile_pool(name="diag", bufs=16))
    out_pool = ctx.enter_context(tc.tile_pool(name="out", bufs=3))
    ps = ctx.enter_context(tc.tile_pool(name="ps", bufs=2, space="PSUM"))
    ps1 = ctx.enter_context(tc.tile_pool(name="ps1", bufs=2, space="PSUM"))

    # ---- whole-core upfront loads (tiny vs mem): h, lengths ----
    h_all = wpool.tile([P, N_TILES, D], BF16)
    nc.sync.dma_start(h_all[:], hw_ap[0:BC, :].rearrange("(t p) d -> p t d", p=P))
    len_ap = sc_ap[SROWS:SROWS + BC]
    lt_all = wpool.tile([P, N_TILES], F16)
    nc.sync.dma_start(lt_all[:], len_ap.rearrange("(t p) -> p t", p=P))
    lrow_all = wpool.tile([1, BC], F16)
    nc.sync.dma_start(lrow_all[:], len_ap.rearrange("(one b) -> one b", one=1))

    # prologue: per-tile +/-BIG softmax masks and empty-row gate masks
    ltf_all = wpool.tile([P, N_TILES], F32)
    nc.vector.tensor_copy(ltf_all[:], lt_all[:])
    maskbig_all = wpool.tile([P, N_TILES, M], F32)
    negrow_all = wpool.tile([1, BC], BF16)
    lrowf_all = wpool.tile([1, BC], F32)
    nc.vector.tensor_copy(lrowf_all[:], lrow_all[:])
    nc.vector.tensor_scalar(negrow_all[:], lrowf_all[:], 0.0, None, Alu.is_gt)
    nc.vector.tensor_scalar(negrow_all[:], negrow_all[:], BIG, BIG, Alu.mult, Alu.subtract)
    for t in range(N_TILES):
        mt_ = tile_caps[t]
        nc.vector.tensor_scalar(
            maskbig_all[:, t, 0:mt_], iota_m[:, 0:mt_], ltf_all[:, t:t + 1],
            None, Alu.is_lt)
        nc.vector.tensor_scalar(
            maskbig_all[:, t, 0:mt_], maskbig_all[:, t, 0:mt_], 2.0 * BIG, BIG,
            Alu.mult, Alu.subtract)

    def scores_front(t):
        """DMA load (packed int8 rows + scales, one DMA per cap group),
        cast, scores, masked softmax, h-transpose."""
        b0 = t * P
        MT = tile_caps[t]
        mq = memq_pool.tile([P, M, D], I8, tag="memq")
        sc16 = small.tile([P, M], F16, tag="sc16")
        # rows above a group's cap are never DMA'd: zero scales neutralize
        # them (score 0 -> masked; exp-weight 0 in the r matmul)
        nc.vector.memset(sc16[:], 0.0)
        for g in range(NG):
            cg = caps[t][g]
            o0 = int(goffs[t * NG + g])
            nc.sync.dma_start(
                mq[g * GP:(g + 1) * GP, 0:cg, :],
                mem_ap[o0:o0 + GP * cg, :].rearrange("(p m) d -> p m d", p=GP),
            )
            nc.sync.dma_start(
                sc16[g * GP:(g + 1) * GP, 0:cg],
                sc_ap[o0:o0 + GP * cg].rearrange("(p m) -> p m", p=GP),
            )
        sc = small.tile([P, M], F32, tag="sc")
        nc.vector.tensor_copy(sc[:, 0:MT], sc16[:, 0:MT])
        # cast q -> bf16 (integers <= 127 are exact in bf16)
        mck = mem_pool.tile([P, M, D], BF16, tag="mem")
        nc.vector.tensor_copy(mck[:, 0:MT, :], mq[:, 0:MT, :])

        ht = h_all[:, t, :]

        # ---- scores[b, m] = <q[b, m, :], h[b, :]> (fused mult+accum) ----
        scratch = small.tile([P, D], BF16, tag="scratch")
        S = small.tile([P, M], F32, tag="S")
        for m in range(MT):
            nc.vector.scalar_tensor_tensor(
                out=scratch[:], in0=mck[:, m, :], scalar=1.0, in1=ht,
                op0=Alu.mult, op1=Alu.mult, accum_out=S[:, m:m + 1],
            )
        # fold the per-row dequant scale into the scores (pre-softmax)
        nc.vector.tensor_tensor(S[:, 0:MT], S[:, 0:MT], sc[:, 0:MT], Alu.mult)

        # ---- masked softmax over m: Sm = min(S, +/-BIG mask) ----
        Sm = small.tile([P, M], F32, tag="Sm")
        nc.vector.tensor_tensor(Sm[:, 0:MT], S[:, 0:MT], maskbig_all[:, t, 0:MT], Alu.min)
        negmax = small.tile([P, 1], F32, tag="negmax")
        nc.vector.tensor_reduce(negmax[:], Sm[:, 0:MT], AX.X, Alu.max, negate=True)
        E = xstage.tile([P, M], F32, tag="E")
        ssum = small.tile([P, 1], F32, tag="ssum")
        # ScalarE accumulator emits the softmax denominator with the exp
        nc.scalar.activation(E[:, 0:MT], Sm[:, 0:MT], Act.Exp, bias=negmax[:],
                             scale=1.0, accum_out=ssum[:])
        rinv = small.tile([P, 1], F32, tag="rinv")
        nc.vector.reciprocal(rinv[:], ssum[:])
        # fold the dequant scale into the attention weights for r = attn @ mem
        nc.vector.tensor_tensor(E[:, 0:MT], E[:, 0:MT], sc[:, 0:MT], Alu.mult)

        # h transpose (only needs ht)
        pt_h = ps1.tile([P, 2, P], BF16, tag="pth")
        hT = xstage.tile([P, 2, P], BF16, tag="hT")
        for k in range(2):
            nc.tensor.transpose(pt_h[:, k, :], ht[:, k * P:(k + 1) * P], ident16[:])
            nc.scalar.copy(hT[:, k, :], pt_h[:, k, :])

        return dict(ht=ht, hT=hT, negrow=negrow_all[:, b0:b0 + P],
                    attn=E, rinv=rinv, mck=mck, b0=b0, MT=MT,
                    last=(t >= N_TILES - 2))

    def r_front(st):
        """r[b, :] = sum_m attn'[b, m] * q[b, m, :], on TensorE via
        diag(attn'_m) bf16 matmuls accumulated in PSUM."""
        attn, mck, MT, last = st["attn"], st["mck"], st["MT"], st["last"]
        R_ps = ps.tile([P, D], F32, tag="Rps")
        for m in range(MT):
            dg = diag_pool.tile([P, P], BF16, tag="dg")
            if last and m % 3 != 0:
                nc.vector.tensor_scalar(dg[:], ident[:], attn[:, m:m + 1], None, Alu.mult)
            else:
                nc.scalar.activation(dg[:], ident[:], Act.Copy, bias=0.0,
                                     scale=attn[:, m:m + 1])
            nc.tensor.matmul(
                R_ps[:], dg[:], mck[:, m, :],
                start=(m == 0), stop=(m == MT - 1),
            )
        st["R_ps"] = R_ps
        return st

    def backend(st):
        """Combine r, gate matmuls, sigmoid, blend, store."""
        ht, R_ps, hT, negrow, b0 = (
            st["ht"], st["R_ps"], st["hT"], st["negrow"], st["b0"]
        )
        R = small.tile([P, D], F32, tag="R")
        nc.scalar.activation(R[:], R_ps[:], Act.Copy, bias=0.0, scale=st["rinv"][:])
        Rb = small.tile([P, D], BF16, tag="Rb")
        nc.vector.tensor_copy(Rb[:], R[:])

        pt_r = ps1.tile([P, 2, P], BF16, tag="ptr")
        rT = small.tile([P, 2, P], BF16, tag="rT")
        for k in range(2):
            nc.tensor.transpose(pt_r[:, k, :], Rb[:, k * P:(k + 1) * P], ident16[:])
            nc.scalar.copy(rT[:, k, :], pt_r[:, k, :])

        # ---- gate preactivation directly in [b, o] layout ----
        # G[b, o] = sum_d hT[d, b] Wg^T[d, o] + sum_d rT[d, b] Ug^T[d, o]
        #           + bias[o] + (-BIG if len_b == 0)
        # (contraction over d: lhsT = hT/rT blocks, rhs = wgT/ugT blocks;
        #  bias and empty-row mask enter as rank-1 matmuls)
        G = ps.tile([P, D], F32, tag="G")
        for ib in range(2):
            nc.tensor.matmul(G[:], hT[:, ib, :], wgT[:, ib, :],
                             start=(ib == 0), stop=False)
        for ib in range(2):
            nc.tensor.matmul(G[:], rT[:, ib, :], ugT[:, ib, :],
                             start=False, stop=False)
        nc.tensor.matmul(G[:], ones_col[:], bias_row[:], start=False, stop=False)
        nc.tensor.matmul(G[:], negrow[:], ones_D[:], start=False, stop=True)

        # y = tanh((pre + bias)/2); g = 0.5*(1+y) folded into the blend.
        g_sb = small.tile([P, D], F32, tag="gT")
        nc.scalar.activation(g_sb[:], G[:], Act.Tanh, bias=0.0, scale=0.5)

        # ---- out = h + 0.5*(1+y)*(r-h) ----
        T1 = small.tile([P, D], F32, tag="T1")
        nc.vector.tensor_tensor(T1[:], R[:], ht, Alu.subtract)
        T2 = small.tile([P, D], F32, tag="T2")
        nc.vector.scalar_tensor_tensor(
            out=T2[:], in0=g_sb[:], scalar=1.0,
            in1=T1[:], op0=Alu.add, op1=Alu.mult,
        )
        ot = out_pool.tile([P, D], BF16, tag="ot")
        nc.vector.scalar_tensor_tensor(
            out=ot[:], in0=T2[:], scalar=0.5, in1=ht, op0=Alu.mult, op1=Alu.add,
        )
        nc.sync.dma_start(out_ap[b0:b0 + P, :], ot[:])

    # 3-stage software pipeline. (REPS>1 is a profiling knob: repeating the
    # whole pipeline isolates device-exec time from transfer time.)
    for _rep in range(REPS):
        stages = []
        for t in range(N_TILES):
            stages.append(scores_front(t))
            if t >= 1:
                r_front(stages[t - 1])
            if t >= 2:
                backend(stages[t - 2])
        r_front(stages[N_TILES - 1])
        backend(stages[N_TILES - 2])
        backend(stages[N_TILES - 1])


_CACHE = {}


def _get_nc(caps):
    key = ("nc", REPS, caps)
    if key in _CACHE:
        return _CACHE[key]
    total_rows = int(GP * sum(c for row in caps for c in row))
    nc = bacc.Bacc("TRN2", target_bir_lowering=False, debug=False, num_devices=N_CORES)
    hw_ap = nc.dram_tensor("h_tilde", [BC + 2 * D + 1, D], BF16,
                           kind="ExternalInput").ap()
    mem_ap = nc.dram_tensor("mem", [total_rows, D], I8, kind="ExternalInput").ap()
    sc_ap = nc.dram_tensor("scales", [total_rows + BC], F16,
                           kind="ExternalInput").ap()
    out_ap = nc.dram_tensor("out", [BC, D], BF16, kind="ExternalOutput").ap()
    io = (hw_ap, mem_ap, sc_ap, out_ap)
    with tile.TileContext(nc) as tc:
        with ExitStack() as ctx:
            _build_body(ctx, tc, io, caps)
    nc.finalize()
    _CACHE[key] = nc
    return nc


def _plan(lengths):
    """Sort samples by length; deal global octile blocks across cores so
    every core has the same per-tile cap profile. Partitions within a tile
    are length-sorted, so each GP-partition group gets its own cap: the max
    of its GP*N_CORES globally-consecutive sorted samples (core-invariant).
    Returns (perm[B] of sample ids in device order core-major,
    caps[N_TILES][NG])."""
    order = np.argsort(lengths, kind="stable")
    caps = []
    perm = np.empty(B, dtype=np.int64)
    for k in range(N_TILES):
        blk = order[k * (P * N_CORES):(k + 1) * (P * N_CORES)]
        caps.append(tuple(
            int(max(1, lengths[blk[g * GP * N_CORES:(g + 1) * GP * N_CORES]].max()))
            for g in range(NG)
        ))
        # core c, tile k, partition p <- blk[p * N_CORES + c]; within each
        # GP-partition cap group the order is free (the group cap bounds the
        # whole 128-sample sorted block), so sort ids numerically there to
        # make the host-side mem gather walk memory sequentially.
        for c in range(N_CORES):
            seg = np.sort(blk[c::N_CORES].reshape(NG, GP), axis=1)
            perm[c * BC + k * P: c * BC + (k + 1) * P] = seg.ravel()
    return perm, tuple(caps)


def _make_in_maps(inputs):
    lengths_full = np.asarray(inputs["lengths"], dtype=np.int32)
    perm, caps = _plan(lengths_full)
    h = np.asarray(inputs["h_tilde"], dtype=np.float32).astype(ml_dtypes.bfloat16)
    mem = np.asarray(inputs["mem"])
    bias = (np.asarray(inputs["Wg_b"], dtype=np.float32)
            + np.asarray(inputs["Ug_b"], dtype=np.float32)
            + np.asarray(inputs["b_g"], dtype=np.float32))
    # replicated rows appended to each core's h block: Wg, Ug, summed bias
    wub = np.concatenate([
        np.asarray(inputs["Wg_w"], dtype=np.float32).astype(ml_dtypes.bfloat16),
        np.asarray(inputs["Ug_w"], dtype=np.float32).astype(ml_dtypes.bfloat16),
        bias.astype(ml_dtypes.bfloat16)[None, :],
    ], axis=0)
    total_rows = int(GP * sum(c for row in caps for c in row))

    in_maps = []
    for c in range(N_CORES):
        ids = perm[c * BC:(c + 1) * BC]
        q_parts = np.empty((total_rows, D), dtype=np.int8)
        s_parts = np.empty((total_rows + BC,), dtype=np.float16)
        s_parts[total_rows:] = lengths_full[ids]  # lengths <= 64: f16-exact
        off = 0
        for k in range(N_TILES):
            for g in range(NG):
                cap = caps[k][g]
                n = GP * cap
                rows = mem[ids[k * P + g * GP:k * P + (g + 1) * GP],
                           :cap, :].reshape(n, D)
                amax = np.maximum(rows.max(axis=1), -rows.min(axis=1))
                np.maximum(amax, 1e-30, out=amax)
                s_parts[off:off + n] = amax * (1.0 / 127.0)
                np.multiply(rows, (127.0 / amax)[:, None], out=rows)
                np.rint(rows, out=q_parts[off:off + n], casting="unsafe")
                off += n
        in_maps.append({
            "h_tilde": np.concatenate([h[ids], wub], axis=0),
            "mem": q_parts,
            "scales": s_parts,
        })
    return in_maps, perm, caps


def run(inputs, **kwargs):
    in_maps, perm, caps = _make_in_maps(inputs)
    nc = _get_nc(caps)
    res = run_bass_kernel_spmd(nc, in_maps, list(range(N_CORES)), **kwargs)
    return res, perm


def kernel(**inputs) -> np.ndarray:
    res, perm = run(inputs)
    permuted = np.concatenate(
        [res.results[c]["out"] for c in range(N_CORES)], axis=0)
    out = np.empty((B, D), dtype=np.float32)
    out[perm] = permuted
    return out
